# revision 4
# baseline (speedup 1.0000x reference)
"""Trainium2 Bass kernel for nn_ConvLocalBlock (Conv1D+BN+ReLU -> LocallyConnected1D+BN+ReLU).

Sharding: sequence-parallel over the L2=504 output positions across 8 cores
(63 positions each), full batch B=256 per core.  Conv weights replicated;
each core computes the y positions (l..l+4 window) it needs locally.

Layouts (host-prepared, fp16 matmul operands, fp32 accumulation):
  x2  [128, 72, 256]  partitions 0:64 = x[c, t], 64:128 = x[c, t+1]  (c-major, b fastest)
  w1t [128, 3, 2, 128] conv weight k-tiles (dt-pairs stacked on partitions), BN1-folded
  lw  [63, 128, 20, 128] per-position local weights as [l, k-part, (kt,oc), m], BN2-folded
  b1  [128, 2]   folded conv bias per (u-part, uc)
  b2  [128, 63, 2] folded local bias per (o-part, l, oc)
Output per core: z [63, 256, 256] fp32 in [l, o, b] layout; host reassembles to [B, L2, U].
"""
import sys
import os

for _p in ('/opt/trn_rl_repo',):
    if _p not in sys.path:
        sys.path.insert(0, _p)

import numpy as np

import concourse.bass as bass
import concourse.tile as tile
import concourse.mybir as mybir
from concourse import bacc, bass_utils

dt = mybir.dt

EPS = 1e-3
FS = 5
B, L, CIN, U = 256, 512, 64, 256
L1 = L - FS + 1            # 508
L2 = L1 - FS + 1           # 504
NCORES = 8
LC = L2 // NCORES          # 63 positions per core
NPOS = LC + FS - 1         # 67 y positions needed per core
XCOLS = NPOS + FS          # 72 x2 columns per core (incl. shifted/zero pad)
KT2 = 10                   # local-stage k tiles (j=0..4  x  uc=0..1)

_NC_CACHE = {}


def _build_nc():
    """Build the single-core Tile program (SPMD across 8 cores)."""
    if 'nc' in _NC_CACHE:
        return _NC_CACHE['nc']
    nc = bacc.Bacc("TRN2", target_bir_lowering=False, debug=False)

    x2_d = nc.dram_tensor("x2", [128, XCOLS * B], dt.float16, kind="ExternalInput")
    w1_d = nc.dram_tensor("w1t", [128, 3 * 2 * 128], dt.float16, kind="ExternalInput")
    lw_d = nc.dram_tensor("lw", [LC, 128, KT2 * 2 * 128], dt.float16, kind="ExternalInput")
    b1_d = nc.dram_tensor("b1", [128, 2], dt.float32, kind="ExternalInput")
    b2_d = nc.dram_tensor("b2", [128, LC * 2], dt.float32, kind="ExternalInput")
    z_d = nc.dram_tensor("z", [LC, U, B], dt.float32, kind="ExternalOutput")

    with tile.TileContext(nc) as tc:
        with tc.tile_pool(name="const", bufs=1) as cpool, \
             tc.tile_pool(name="ybuf", bufs=1) as ypool, \
             tc.tile_pool(name="lwp", bufs=3) as lwpool, \
             tc.tile_pool(name="zp", bufs=4) as zpool, \
             tc.tile_pool(name="ps1", bufs=3, space="PSUM") as ps1, \
             tc.tile_pool(name="ps2", bufs=4, space="PSUM") as ps2:

            x2_t = cpool.tile([128, XCOLS, B], dt.float16)
            w1_t = cpool.tile([128, 3, 2, 128], dt.float16)
            b1_t = cpool.tile([128, 2], dt.float32)
            b2_t = cpool.tile([128, LC, 2], dt.float32)
            nc.sync.dma_start(x2_t[:], x2_d.ap().rearrange("p (t b) -> p t b", b=B))
            nc.sync.dma_start(w1_t[:], w1_d.ap().rearrange("p (k u m) -> p k u m", k=3, u=2))
            nc.sync.dma_start(b1_t[:], b1_d.ap()[:])
            nc.sync.dma_start(b2_t[:], b2_d.ap().rearrange("p (l u) -> p l u", u=2))

            y_t = [ypool.tile([128, NPOS * B], dt.float16, tag=f"y{uc}",
                              name=f"y{uc}") for uc in range(2)]

            # ---- stage 1: conv (+BN1+ReLU) into Y[uc][:, t*B : (t+2)*B] ----
            ngroups = (NPOS + 1) // 2          # 34 groups (last single-position)
            for g in range(ngroups):
                npos_g = 2 if 2 * g + 1 < NPOS else 1
                n = npos_g * B
                t0 = 2 * g
                for uc in range(2):
                    ps = ps1.tile([128, 2 * B], dt.float32, tag="convps")
                    for kt in range(3):
                        # k-tile kt reads x2 columns shifted by 2*kt
                        rhs = x2_t[:, t0 + 2 * kt: t0 + 2 * kt + npos_g, :]
                        if kt == 2:
                            rhs = x2_t[:64, t0 + 4: t0 + 4 + npos_g, :]
                            lhsT = w1_t[:64, 2, uc, :]
                        else:
                            lhsT = w1_t[:, kt, uc, :]
                        nc.tensor.matmul(ps[:, :n], lhsT, rhs,
                                         start=(kt == 0), stop=(kt == 2))
                    # BN1+ReLU epilogue on DVE: relu(x + b1) with fp16 output
                    nc.vector.tensor_scalar(
                        out=y_t[uc][:, t0 * B: t0 * B + n],
                        in0=ps[:, :n],
                        scalar1=b1_t[:, uc:uc + 1],
                        scalar2=0.0,
                        op0=mybir.AluOpType.add,
                        op1=mybir.AluOpType.max)

            # ---- stage 2: locally-connected (+BN2+ReLU) ----
            for l in range(LC):
                lw_t = lwpool.tile([128, KT2 * 2, 128], dt.float16, tag="lw")
                nc.sync.dma_start(
                    lw_t[:], lw_d.ap()[l].rearrange("p (k m) -> p k m", m=128))
                for oc in range(2):
                    ps = ps2.tile([128, B], dt.float32, tag="locps")
                    for kt in range(KT2):
                        j, uc = kt // 2, kt % 2
                        nc.tensor.matmul(
                            ps[:],
                            lw_t[:, kt * 2 + oc, :],
                            y_t[uc][:, (l + j) * B: (l + j + 1) * B],
                            start=(kt == 0), stop=(kt == KT2 - 1))
                    z_sb = zpool.tile([128, B], dt.float32, tag="z")
                    nc.scalar.activation(
                        z_sb[:], ps[:], mybir.ActivationFunctionType.Relu,
                        bias=b2_t[:, l, oc:oc + 1], scale=1.0)
                    nc.sync.dma_start(z_d.ap()[l, oc * 128:(oc + 1) * 128, :], z_sb[:])

    nc.compile()
    _NC_CACHE['nc'] = nc
    return nc


def _preprocess(x, conv_w, conv_b, g1, b1, m1, v1, local_w, local_b, g2, b2, m2, v2):
    """Fold BN into weights/biases, build per-core shards in device layouts."""
    f32 = np.float32
    a1 = (g1 / np.sqrt(v1 + EPS)).astype(f32)                      # [U]
    bias1 = ((conv_b - m1) * a1 + b1).astype(f32)                  # [U]
    a2 = (g2 / np.sqrt(v2 + EPS)).astype(f32)                      # [U]
    bias2 = ((local_b - m2[None, :]) * a2[None, :] + b2[None, :]).astype(f32)  # [L2, U]

    w1f = (conv_w * a1[None, None, :]).astype(np.float16)          # [5, 64, 256]
    w1r = w1f.reshape(FS, CIN, 2, 128)                             # [dt, c, uc, m]
    w1t = np.zeros((128, 3, 2, 128), np.float16)
    for kt in range(3):
        w1t[0:64, kt] = w1r[2 * kt]
        if 2 * kt + 1 < FS:
            w1t[64:128, kt] = w1r[2 * kt + 1]

    # local weights: [L2, 1280, 256] * a2 -> fp16 -> [core, l, p, kt*2+oc, m]
    lwf = (local_w * a2[None, None, :]).astype(np.float16)
    lwp = lwf.reshape(NCORES, LC, KT2, 128, 2, 128).transpose(0, 1, 3, 2, 4, 5)
    lwp = np.ascontiguousarray(lwp)            # [core, l, p, kt, oc, m]

    # x2: [128, 513, 256] fp16; top=x[c,t], bottom=x[c,t+1]
    xt = np.ascontiguousarray(x.transpose(2, 1, 0)).astype(np.float16)  # [c, t, b]
    x2g = np.zeros((128, L + 1, B), np.float16)
    x2g[0:64, 0:L] = xt
    x2g[64:128, 0:L - 1] = xt[:, 1:L]

    b1_sb = np.ascontiguousarray(bias1.reshape(2, 128).T)          # [p, uc]
    b2_all = bias2.reshape(NCORES, LC, 2, 128).transpose(0, 3, 1, 2)  # [core, p, l, oc]

    in_maps = []
    for c in range(NCORES):
        t0 = LC * c
        x2_c = np.ascontiguousarray(x2g[:, t0: t0 + XCOLS]).reshape(128, XCOLS * B)
        in_maps.append({
            "x2": x2_c,
            "w1t": np.ascontiguousarray(w1t).reshape(128, 3 * 2 * 128),
            "lw": np.ascontiguousarray(lwp[c]).reshape(LC, 128, KT2 * 2 * 128),
            "b1": b1_sb,
            "b2": np.ascontiguousarray(b2_all[c]).reshape(128, LC * 2),
        })
    return in_maps


def kernel(**inputs):
    nc = _build_nc()
    in_maps = _preprocess(**inputs)
    trace = bool(int(os.environ.get("BASS_KERNEL_TRACE", "0")))
    res = bass_utils.run_bass_kernel_spmd(
        nc, in_maps, core_ids=list(range(NCORES)), trace=trace)
    if trace:
        kernel.last_exec_time_ns = res.exec_time_ns
        kernel.last_results = res
    out = np.empty((B, L2, U), np.float32)
    for c in range(NCORES):
        z = res.results[c]["z"].reshape(LC, U, B)
        out[:, LC * c: LC * (c + 1), :] = z.transpose(2, 0, 1)
    return out


# revision 5
# speedup vs baseline: 1.2524x; 1.2524x over previous
"""Trainium2 Bass kernel for nn_ConvLocalBlock (Conv1D+BN+ReLU -> LocallyConnected1D+BN+ReLU).

Sharding: sequence-parallel over the L2=504 output positions across 8 cores
(63 positions each), full batch B=256 per core.  Conv weights replicated;
each core computes the y positions (l..l+4 window) it needs locally.

Layouts (host-prepared, fp16 matmul operands, fp32 accumulation):
  x2  [128, 72, 256]  partitions 0:64 = x[c, t], 64:128 = x[c, t+1]  (c-major, b fastest)
  w1t [128, 3, 2, 128] conv weight k-tiles (dt-pairs stacked on partitions), BN1-folded
  lw  [63, 128, 20, 128] per-position local weights as [l, k-part, (kt,oc), m], BN2-folded
  b1  [128, 2]   folded conv bias per (u-part, uc)
  b2  [128, 63, 2] folded local bias per (o-part, l, oc)
Output per core: z [63, 256, 256] fp32 in [l, o, b] layout; host reassembles to [B, L2, U].
"""
import sys
import os

for _p in ('/opt/trn_rl_repo',):
    if _p not in sys.path:
        sys.path.insert(0, _p)

import numpy as np

import concourse.bass as bass
import concourse.tile as tile
import concourse.mybir as mybir
from concourse import bacc, bass_utils

dt = mybir.dt

EPS = 1e-3
FS = 5
B, L, CIN, U = 256, 512, 64, 256
L1 = L - FS + 1            # 508
L2 = L1 - FS + 1           # 504
NCORES = 8
LC = L2 // NCORES          # 63 positions per core
NPOS = LC + FS - 1         # 67 y positions needed per core
XCOLS = NPOS + FS          # 72 x2 columns per core (incl. shifted/zero pad)
KT2 = 10                   # local-stage k tiles (j=0..4  x  uc=0..1)

_NC_CACHE = {}


def _build_nc():
    """Build the single-core Tile program (SPMD across 8 cores)."""
    if 'nc' in _NC_CACHE:
        return _NC_CACHE['nc']
    nc = bacc.Bacc("TRN2", target_bir_lowering=False, debug=False)

    x2_d = nc.dram_tensor("x2", [128, XCOLS * B], dt.float16, kind="ExternalInput")
    w1_d = nc.dram_tensor("w1t", [128, 3 * 2 * 128], dt.float16, kind="ExternalInput")
    lw_d = nc.dram_tensor("lw", [LC, 128, KT2 * 2 * 128], dt.float16, kind="ExternalInput")
    b1_d = nc.dram_tensor("b1", [128, 2], dt.float32, kind="ExternalInput")
    b2_d = nc.dram_tensor("b2", [128, LC * 2], dt.float32, kind="ExternalInput")
    z_d = nc.dram_tensor("z", [LC, U, B], dt.float32, kind="ExternalOutput")

    with tile.TileContext(nc) as tc:
        with tc.tile_pool(name="const", bufs=1) as cpool, \
             tc.tile_pool(name="ybuf", bufs=1) as ypool, \
             tc.tile_pool(name="lwp", bufs=8) as lwpool, \
             tc.tile_pool(name="zp", bufs=6) as zpool, \
             tc.tile_pool(name="ps1", bufs=4, space="PSUM") as ps1, \
             tc.tile_pool(name="ps2", bufs=4, space="PSUM") as ps2:

            x2_t = cpool.tile([128, XCOLS, B], dt.float16)
            w1_t = cpool.tile([128, 3, 2, 128], dt.float16)
            b1_t = cpool.tile([128, 2], dt.float32)
            b2_t = cpool.tile([128, LC, 2], dt.float32)
            x2_src = x2_d.ap().rearrange("p (t b) -> p t b", b=B)
            for cb in range(0, XCOLS, 18):
                ce = min(cb + 18, XCOLS)
                nc.sync.dma_start(x2_t[:, cb:ce, :], x2_src[:, cb:ce, :])
            nc.sync.dma_start(w1_t[:], w1_d.ap().rearrange("p (k u m) -> p k u m", k=3, u=2))
            nc.sync.dma_start(b1_t[:], b1_d.ap()[:])
            nc.sync.dma_start(b2_t[:], b2_d.ap().rearrange("p (l u) -> p l u", u=2))

            y_t = [ypool.tile([128, NPOS * B], dt.float16, tag=f"y{uc}",
                              name=f"y{uc}") for uc in range(2)]

            # ---- stage 1: conv (+BN1+ReLU) into Y[uc][:, t*B : (t+2)*B] ----
            ngroups = (NPOS + 1) // 2          # 34 groups (last single-position)
            for g in range(ngroups):
                npos_g = 2 if 2 * g + 1 < NPOS else 1
                n = npos_g * B
                t0 = 2 * g
                for uc in range(2):
                    ps = ps1.tile([128, 2 * B], dt.float32, tag="convps")
                    for kt in range(3):
                        # k-tile kt reads x2 columns shifted by 2*kt
                        rhs = x2_t[:, t0 + 2 * kt: t0 + 2 * kt + npos_g, :]
                        if kt == 2:
                            rhs = x2_t[:64, t0 + 4: t0 + 4 + npos_g, :]
                            lhsT = w1_t[:64, 2, uc, :]
                        else:
                            lhsT = w1_t[:, kt, uc, :]
                        nc.tensor.matmul(ps[:, :n], lhsT, rhs,
                                         start=(kt == 0), stop=(kt == 2))
                    # BN1+ReLU epilogue: relu(x + b1), fp16 out; DVE for uc0, ACT for uc1
                    if uc == 0:
                        nc.vector.tensor_scalar(
                            out=y_t[uc][:, t0 * B: t0 * B + n],
                            in0=ps[:, :n],
                            scalar1=b1_t[:, uc:uc + 1],
                            scalar2=0.0,
                            op0=mybir.AluOpType.add,
                            op1=mybir.AluOpType.max)
                    else:
                        nc.scalar.activation(
                            y_t[uc][:, t0 * B: t0 * B + n], ps[:, :n],
                            mybir.ActivationFunctionType.Relu,
                            bias=b1_t[:, uc:uc + 1], scale=1.0)

            # ---- stage 2: locally-connected (+BN2+ReLU) ----
            for l in range(LC):
                lw_t = lwpool.tile([128, KT2 * 2, 128], dt.float16, tag="lw")
                nc.sync.dma_start(
                    lw_t[:], lw_d.ap()[l].rearrange("p (k m) -> p k m", m=128))
                for oc in range(2):
                    ps = ps2.tile([128, B], dt.float32, tag="locps")
                    for kt in range(KT2):
                        j, uc = kt // 2, kt % 2
                        nc.tensor.matmul(
                            ps[:],
                            lw_t[:, kt * 2 + oc, :],
                            y_t[uc][:, (l + j) * B: (l + j + 1) * B],
                            start=(kt == 0), stop=(kt == KT2 - 1))
                    z_sb = zpool.tile([128, B], dt.float32, tag="z")
                    nc.scalar.activation(
                        z_sb[:], ps[:], mybir.ActivationFunctionType.Relu,
                        bias=b2_t[:, l, oc:oc + 1], scale=1.0)
                    nc.scalar.dma_start(z_d.ap()[l, oc * 128:(oc + 1) * 128, :], z_sb[:])

    nc.compile()
    _NC_CACHE['nc'] = nc
    return nc


def _preprocess(x, conv_w, conv_b, g1, b1, m1, v1, local_w, local_b, g2, b2, m2, v2):
    """Fold BN into weights/biases, build per-core shards in device layouts."""
    f32 = np.float32
    a1 = (g1 / np.sqrt(v1 + EPS)).astype(f32)                      # [U]
    bias1 = ((conv_b - m1) * a1 + b1).astype(f32)                  # [U]
    a2 = (g2 / np.sqrt(v2 + EPS)).astype(f32)                      # [U]
    bias2 = ((local_b - m2[None, :]) * a2[None, :] + b2[None, :]).astype(f32)  # [L2, U]

    w1f = (conv_w * a1[None, None, :]).astype(np.float16)          # [5, 64, 256]
    w1r = w1f.reshape(FS, CIN, 2, 128)                             # [dt, c, uc, m]
    w1t = np.zeros((128, 3, 2, 128), np.float16)
    for kt in range(3):
        w1t[0:64, kt] = w1r[2 * kt]
        if 2 * kt + 1 < FS:
            w1t[64:128, kt] = w1r[2 * kt + 1]

    # local weights: [L2, 1280, 256] * a2 -> fp16 -> [core, l, p, kt*2+oc, m]
    lwf = (local_w * a2[None, None, :]).astype(np.float16)
    lwp = lwf.reshape(NCORES, LC, KT2, 128, 2, 128).transpose(0, 1, 3, 2, 4, 5)
    lwp = np.ascontiguousarray(lwp)            # [core, l, p, kt, oc, m]

    # x2: [128, 513, 256] fp16; top=x[c,t], bottom=x[c,t+1]
    xt = np.ascontiguousarray(x.transpose(2, 1, 0)).astype(np.float16)  # [c, t, b]
    x2g = np.zeros((128, L + 1, B), np.float16)
    x2g[0:64, 0:L] = xt
    x2g[64:128, 0:L - 1] = xt[:, 1:L]

    b1_sb = np.ascontiguousarray(bias1.reshape(2, 128).T)          # [p, uc]
    b2_all = bias2.reshape(NCORES, LC, 2, 128).transpose(0, 3, 1, 2)  # [core, p, l, oc]

    in_maps = []
    for c in range(NCORES):
        t0 = LC * c
        x2_c = np.ascontiguousarray(x2g[:, t0: t0 + XCOLS]).reshape(128, XCOLS * B)
        in_maps.append({
            "x2": x2_c,
            "w1t": np.ascontiguousarray(w1t).reshape(128, 3 * 2 * 128),
            "lw": np.ascontiguousarray(lwp[c]).reshape(LC, 128, KT2 * 2 * 128),
            "b1": b1_sb,
            "b2": np.ascontiguousarray(b2_all[c]).reshape(128, LC * 2),
        })
    return in_maps


def kernel(**inputs):
    nc = _build_nc()
    in_maps = _preprocess(**inputs)
    trace = bool(int(os.environ.get("BASS_KERNEL_TRACE", "0")))
    res = bass_utils.run_bass_kernel_spmd(
        nc, in_maps, core_ids=list(range(NCORES)), trace=trace)
    if trace:
        kernel.last_exec_time_ns = res.exec_time_ns
        kernel.last_results = res
    out = np.empty((B, L2, U), np.float32)
    for c in range(NCORES):
        z = res.results[c]["z"].reshape(LC, U, B)
        out[:, LC * c: LC * (c + 1), :] = z.transpose(2, 0, 1)
    return out


# revision 7
# speedup vs baseline: 1.5295x; 1.2212x over previous
"""Trainium2 Bass kernel for nn_ConvLocalBlock (Conv1D+BN+ReLU -> LocallyConnected1D+BN+ReLU).

Sharding: sequence-parallel over the L2=504 output positions across 8 cores
(63 positions each), full batch B=256 per core.  Conv weights replicated;
each core computes the y positions (l..l+4 window) it needs locally.

Layouts (host-prepared, fp16 matmul operands, fp32 accumulation):
  x2  [128, 72, 256]  partitions 0:64 = x[c, t], 64:128 = x[c, t+1]  (c-major, b fastest)
  w1t [128, 3, 2, 128] conv weight k-tiles (dt-pairs stacked on partitions), BN1-folded
  lw  [63, 128, 20, 128] per-position local weights as [l, k-part, (kt,oc), m], BN2-folded
  b1  [128, 2]   folded conv bias per (u-part, uc)
  b2  [128, 63, 2] folded local bias per (o-part, l, oc)
Output per core: z [63, 256, 256] fp32 in [l, o, b] layout; host reassembles to [B, L2, U].
"""
import sys
import os

for _p in ('/opt/trn_rl_repo',):
    if _p not in sys.path:
        sys.path.insert(0, _p)

import numpy as np

import concourse.bass as bass
import concourse.tile as tile
import concourse.mybir as mybir
from concourse import bacc, bass_utils

dt = mybir.dt

EPS = 1e-3
FS = 5
B, L, CIN, U = 256, 512, 64, 256
L1 = L - FS + 1            # 508
L2 = L1 - FS + 1           # 504
NCORES = 8
LC = L2 // NCORES          # 63 positions per core
NPOS = LC + FS - 1         # 67 y positions needed per core
XCOLS = NPOS + FS          # 72 x2 columns per core (incl. shifted/zero pad)
KT2 = 10                   # local-stage k tiles (j=0..4  x  uc=0..1)

_NC_CACHE = {}


def _build_nc():
    """Build the single-core Tile program (SPMD across 8 cores)."""
    if 'nc' in _NC_CACHE:
        return _NC_CACHE['nc']
    nc = bacc.Bacc("TRN2", target_bir_lowering=False, debug=False)

    x2_d = nc.dram_tensor("x2", [128, XCOLS * B], dt.float16, kind="ExternalInput")
    w1_d = nc.dram_tensor("w1t", [128, 3 * 2 * 128], dt.float16, kind="ExternalInput")
    lw_d = nc.dram_tensor("lw", [LC, 128, KT2 * 2 * 128], dt.float16, kind="ExternalInput")
    b1_d = nc.dram_tensor("b1", [128, 2], dt.float32, kind="ExternalInput")
    b2_d = nc.dram_tensor("b2", [128, LC * 2], dt.float32, kind="ExternalInput")
    z_d = nc.dram_tensor("z", [LC, U, B], dt.float32, kind="ExternalOutput")

    with tile.TileContext(nc) as tc:
        with tc.tile_pool(name="const", bufs=1) as cpool, \
             tc.tile_pool(name="ybuf", bufs=1) as ypool, \
             tc.tile_pool(name="lwp", bufs=8) as lwpool, \
             tc.tile_pool(name="zp", bufs=6) as zpool, \
             tc.tile_pool(name="ps1", bufs=4, space="PSUM") as ps1, \
             tc.tile_pool(name="ps2", bufs=4, space="PSUM") as ps2:

            x2_t = cpool.tile([128, XCOLS, B], dt.float16)
            w1_t = cpool.tile([128, 3, 2, 128], dt.float16)
            b1_t = cpool.tile([128, 2], dt.float32)
            b2_t = cpool.tile([128, LC, 2], dt.float32)
            nc.sync.dma_start(w1_t[:], w1_d.ap().rearrange("p (k u m) -> p k u m", k=3, u=2))
            nc.sync.dma_start(b1_t[:], b1_d.ap()[:])
            nc.sync.dma_start(b2_t[:], b2_d.ap().rearrange("p (l u) -> p l u", u=2))
            x2_src = x2_d.ap().rearrange("p (t b) -> p t b", b=B)
            for cb in range(0, XCOLS, 18):
                ce = min(cb + 18, XCOLS)
                nc.sync.dma_start(x2_t[:, cb:ce, :], x2_src[:, cb:ce, :])

            y_t = [ypool.tile([128, NPOS * B], dt.float16, tag=f"y{uc}",
                              name=f"y{uc}") for uc in range(2)]

            # ---- stage 1: conv (+BN1+ReLU) into Y[uc][:, t*B : (t+2)*B] ----
            ngroups = (NPOS + 1) // 2          # 34 groups (last single-position)
            for g in range(ngroups):
                npos_g = 2 if 2 * g + 1 < NPOS else 1
                n = npos_g * B
                t0 = 2 * g
                for uc in range(2):
                    ps = ps1.tile([128, 2 * B], dt.float32, tag="convps")
                    for kt in range(3):
                        # k-tile kt reads x2 columns shifted by 2*kt; kt2's
                        # lower 64 weight rows are zero (host-padded), keeping
                        # every matmul a uniform full-array K=128 op.
                        rhs = x2_t[:, t0 + 2 * kt: t0 + 2 * kt + npos_g, :]
                        lhsT = w1_t[:, kt, uc, :]
                        nc.tensor.matmul(ps[:, :n], lhsT, rhs,
                                         start=(kt == 0), stop=(kt == 2))
                    # BN1+ReLU epilogue: relu(x + b1), fp16 out; DVE for uc0, ACT for uc1
                    if uc == 0:
                        nc.vector.tensor_scalar(
                            out=y_t[uc][:, t0 * B: t0 * B + n],
                            in0=ps[:, :n],
                            scalar1=b1_t[:, uc:uc + 1],
                            scalar2=0.0,
                            op0=mybir.AluOpType.add,
                            op1=mybir.AluOpType.max)
                    else:
                        nc.scalar.activation(
                            y_t[uc][:, t0 * B: t0 * B + n], ps[:, :n],
                            mybir.ActivationFunctionType.Relu,
                            bias=b1_t[:, uc:uc + 1], scale=1.0)

            # ---- stage 2: locally-connected (+BN2+ReLU) ----
            for l in range(LC):
                lw_t = lwpool.tile([128, KT2 * 2, 128], dt.float16, tag="lw")
                nc.sync.dma_start(
                    lw_t[:], lw_d.ap()[l].rearrange("p (k m) -> p k m", m=128))
                for oc in range(2):
                    ps = ps2.tile([128, B], dt.float32, tag="locps")
                    for kt in range(KT2):
                        j, uc = kt // 2, kt % 2
                        nc.tensor.matmul(
                            ps[:],
                            lw_t[:, kt * 2 + oc, :],
                            y_t[uc][:, (l + j) * B: (l + j + 1) * B],
                            start=(kt == 0), stop=(kt == KT2 - 1))
                    z_sb = zpool.tile([128, B], dt.float32, tag="z")
                    nc.scalar.activation(
                        z_sb[:], ps[:], mybir.ActivationFunctionType.Relu,
                        bias=b2_t[:, l, oc:oc + 1], scale=1.0)
                    nc.scalar.dma_start(z_d.ap()[l, oc * 128:(oc + 1) * 128, :], z_sb[:])

    nc.compile()
    _NC_CACHE['nc'] = nc
    return nc


def _preprocess(x, conv_w, conv_b, g1, b1, m1, v1, local_w, local_b, g2, b2, m2, v2):
    """Fold BN into weights/biases, build per-core shards in device layouts."""
    f32 = np.float32
    a1 = (g1 / np.sqrt(v1 + EPS)).astype(f32)                      # [U]
    bias1 = ((conv_b - m1) * a1 + b1).astype(f32)                  # [U]
    a2 = (g2 / np.sqrt(v2 + EPS)).astype(f32)                      # [U]
    bias2 = ((local_b - m2[None, :]) * a2[None, :] + b2[None, :]).astype(f32)  # [L2, U]

    w1f = (conv_w * a1[None, None, :]).astype(np.float16)          # [5, 64, 256]
    w1r = w1f.reshape(FS, CIN, 2, 128)                             # [dt, c, uc, m]
    w1t = np.zeros((128, 3, 2, 128), np.float16)
    for kt in range(3):
        w1t[0:64, kt] = w1r[2 * kt]
        if 2 * kt + 1 < FS:
            w1t[64:128, kt] = w1r[2 * kt + 1]

    # local weights: [L2, 1280, 256] * a2 -> fp16 -> [core, l, p, kt*2+oc, m]
    lwf = (local_w * a2[None, None, :]).astype(np.float16)
    lwp = lwf.reshape(NCORES, LC, KT2, 128, 2, 128).transpose(0, 1, 3, 2, 4, 5)
    lwp = np.ascontiguousarray(lwp)            # [core, l, p, kt, oc, m]

    # x2: [128, 513, 256] fp16; top=x[c,t], bottom=x[c,t+1]
    xt = np.ascontiguousarray(x.transpose(2, 1, 0)).astype(np.float16)  # [c, t, b]
    x2g = np.zeros((128, L + 1, B), np.float16)
    x2g[0:64, 0:L] = xt
    x2g[64:128, 0:L - 1] = xt[:, 1:L]

    b1_sb = np.ascontiguousarray(bias1.reshape(2, 128).T)          # [p, uc]
    b2_all = bias2.reshape(NCORES, LC, 2, 128).transpose(0, 3, 1, 2)  # [core, p, l, oc]

    in_maps = []
    for c in range(NCORES):
        t0 = LC * c
        x2_c = np.ascontiguousarray(x2g[:, t0: t0 + XCOLS]).reshape(128, XCOLS * B)
        in_maps.append({
            "x2": x2_c,
            "w1t": np.ascontiguousarray(w1t).reshape(128, 3 * 2 * 128),
            "lw": np.ascontiguousarray(lwp[c]).reshape(LC, 128, KT2 * 2 * 128),
            "b1": b1_sb,
            "b2": np.ascontiguousarray(b2_all[c]).reshape(128, LC * 2),
        })
    return in_maps


def kernel(**inputs):
    nc = _build_nc()
    in_maps = _preprocess(**inputs)
    trace = bool(int(os.environ.get("BASS_KERNEL_TRACE", "0")))
    res = bass_utils.run_bass_kernel_spmd(
        nc, in_maps, core_ids=list(range(NCORES)), trace=trace)
    if trace:
        kernel.last_exec_time_ns = res.exec_time_ns
        kernel.last_results = res
    out = np.empty((B, L2, U), np.float32)
    for c in range(NCORES):
        z = res.results[c]["z"].reshape(LC, U, B)
        out[:, LC * c: LC * (c + 1), :] = z.transpose(2, 0, 1)
    return out


# revision 8
# speedup vs baseline: 1.5636x; 1.0223x over previous
"""Trainium2 Bass kernel for nn_ConvLocalBlock (Conv1D+BN+ReLU -> LocallyConnected1D+BN+ReLU).

Sharding: sequence-parallel over the L2=504 output positions across 8 cores
(63 positions each), full batch B=256 per core.  Conv weights replicated;
each core computes the y positions (l..l+4 window) it needs locally.

Layouts (host-prepared, fp16 matmul operands, fp32 accumulation):
  x2  [128, 72, 256]  partitions 0:64 = x[c, t], 64:128 = x[c, t+1]  (c-major, b fastest)
  w1t [128, 3, 2, 128] conv weight k-tiles (dt-pairs stacked on partitions), BN1-folded
  lw  [63, 128, 20, 128] per-position local weights as [l, k-part, (kt,oc), m], BN2-folded
  b1  [128, 2]   folded conv bias per (u-part, uc)
  b2  [128, 63, 2] folded local bias per (o-part, l, oc)
Output per core: z [63, 256, 256] fp32 in [l, o, b] layout; host reassembles to [B, L2, U].
"""
import sys
import os

for _p in ('/opt/trn_rl_repo',):
    if _p not in sys.path:
        sys.path.insert(0, _p)

import numpy as np

import concourse.bass as bass
import concourse.tile as tile
import concourse.mybir as mybir
from concourse import bacc, bass_utils

dt = mybir.dt

EPS = 1e-3
FS = 5
B, L, CIN, U = 256, 512, 64, 256
L1 = L - FS + 1            # 508
L2 = L1 - FS + 1           # 504
NCORES = 8
LC = L2 // NCORES          # 63 positions per core
NPOS = LC + FS - 1         # 67 y positions needed per core
XCOLS = NPOS + FS          # 72 x2 columns per core (incl. shifted/zero pad)
KT2 = 10                   # local-stage k tiles (j=0..4  x  uc=0..1)

_NC_CACHE = {}


def _build_nc():
    """Build the single-core Tile program (SPMD across 8 cores)."""
    if 'nc' in _NC_CACHE:
        return _NC_CACHE['nc']
    nc = bacc.Bacc("TRN2", target_bir_lowering=False, debug=False)

    x2_d = nc.dram_tensor("x2", [128, XCOLS * B], dt.float16, kind="ExternalInput")
    w1_d = nc.dram_tensor("w1t", [128, 3 * 2 * 128], dt.float16, kind="ExternalInput")
    lw_d = nc.dram_tensor("lw", [LC, 128, KT2 * 2 * 128], dt.float16, kind="ExternalInput")
    b1_d = nc.dram_tensor("b1", [128, 2], dt.float32, kind="ExternalInput")
    b2_d = nc.dram_tensor("b2", [128, LC * 2], dt.float32, kind="ExternalInput")
    z_d = nc.dram_tensor("z", [LC, U, B], dt.float16, kind="ExternalOutput")

    with tile.TileContext(nc) as tc:
        with tc.tile_pool(name="const", bufs=1) as cpool, \
             tc.tile_pool(name="ybuf", bufs=1) as ypool, \
             tc.tile_pool(name="lwp", bufs=8) as lwpool, \
             tc.tile_pool(name="zp", bufs=6) as zpool, \
             tc.tile_pool(name="ps1", bufs=4, space="PSUM") as ps1, \
             tc.tile_pool(name="ps2", bufs=4, space="PSUM") as ps2:

            x2_t = cpool.tile([128, XCOLS, B], dt.float16)
            w1_t = cpool.tile([128, 3, 2, 128], dt.float16)
            b1_t = cpool.tile([128, 2], dt.float32)
            b2_t = cpool.tile([128, LC, 2], dt.float32)
            nc.sync.dma_start(w1_t[:], w1_d.ap().rearrange("p (k u m) -> p k u m", k=3, u=2))
            x2_src = x2_d.ap().rearrange("p (t b) -> p t b", b=B)
            x2_bounds = [0, 6] + list(range(18, XCOLS, 12)) + [XCOLS]
            nc.sync.dma_start(x2_t[:, 0:6, :], x2_src[:, 0:6, :])
            nc.sync.dma_start(b1_t[:], b1_d.ap()[:])
            for cb, ce in zip(x2_bounds[1:-1], x2_bounds[2:]):
                nc.sync.dma_start(x2_t[:, cb:ce, :], x2_src[:, cb:ce, :])
            nc.sync.dma_start(b2_t[:], b2_d.ap().rearrange("p (l u) -> p l u", u=2))

            y_t = [ypool.tile([128, NPOS * B], dt.float16, tag=f"y{uc}",
                              name=f"y{uc}") for uc in range(2)]

            # ---- stage 1: conv (+BN1+ReLU) into Y[uc][:, t*B : (t+2)*B] ----
            ngroups = (NPOS + 1) // 2          # 34 groups (last single-position)
            for g in range(ngroups):
                npos_g = 2 if 2 * g + 1 < NPOS else 1
                n = npos_g * B
                t0 = 2 * g
                for uc in range(2):
                    ps = ps1.tile([128, 2 * B], dt.float32, tag="convps")
                    for kt in range(3):
                        # k-tile kt reads x2 columns shifted by 2*kt; kt2's
                        # lower 64 weight rows are zero (host-padded), keeping
                        # every matmul a uniform full-array K=128 op.
                        rhs = x2_t[:, t0 + 2 * kt: t0 + 2 * kt + npos_g, :]
                        lhsT = w1_t[:, kt, uc, :]
                        nc.tensor.matmul(ps[:, :n], lhsT, rhs,
                                         start=(kt == 0), stop=(kt == 2))
                    # BN1+ReLU epilogue: relu(x + b1), fp16 out; DVE for uc0, ACT for uc1
                    if uc == 0:
                        nc.vector.tensor_scalar(
                            out=y_t[uc][:, t0 * B: t0 * B + n],
                            in0=ps[:, :n],
                            scalar1=b1_t[:, uc:uc + 1],
                            scalar2=0.0,
                            op0=mybir.AluOpType.add,
                            op1=mybir.AluOpType.max)
                    else:
                        nc.scalar.activation(
                            y_t[uc][:, t0 * B: t0 * B + n], ps[:, :n],
                            mybir.ActivationFunctionType.Relu,
                            bias=b1_t[:, uc:uc + 1], scale=1.0)

            # ---- stage 2: locally-connected (+BN2+ReLU) ----
            for l in range(LC):
                lw_t = lwpool.tile([128, KT2 * 2, 128], dt.float16, tag="lw")
                nc.sync.dma_start(
                    lw_t[:], lw_d.ap()[l].rearrange("p (k m) -> p k m", m=128))
                for oc in range(2):
                    ps = ps2.tile([128, B], dt.float32, tag="locps")
                    for kt in range(KT2):
                        j, uc = kt // 2, kt % 2
                        nc.tensor.matmul(
                            ps[:],
                            lw_t[:, kt * 2 + oc, :],
                            y_t[uc][:, (l + j) * B: (l + j + 1) * B],
                            start=(kt == 0), stop=(kt == KT2 - 1))
                    z_sb = zpool.tile([128, B], dt.float16, tag="z")
                    nc.scalar.activation(
                        z_sb[:], ps[:], mybir.ActivationFunctionType.Relu,
                        bias=b2_t[:, l, oc:oc + 1], scale=1.0)
                    nc.scalar.dma_start(z_d.ap()[l, oc * 128:(oc + 1) * 128, :], z_sb[:])

    nc.compile()
    _NC_CACHE['nc'] = nc
    return nc


def _preprocess(x, conv_w, conv_b, g1, b1, m1, v1, local_w, local_b, g2, b2, m2, v2):
    """Fold BN into weights/biases, build per-core shards in device layouts."""
    f32 = np.float32
    a1 = (g1 / np.sqrt(v1 + EPS)).astype(f32)                      # [U]
    bias1 = ((conv_b - m1) * a1 + b1).astype(f32)                  # [U]
    a2 = (g2 / np.sqrt(v2 + EPS)).astype(f32)                      # [U]
    bias2 = ((local_b - m2[None, :]) * a2[None, :] + b2[None, :]).astype(f32)  # [L2, U]

    w1f = (conv_w * a1[None, None, :]).astype(np.float16)          # [5, 64, 256]
    w1r = w1f.reshape(FS, CIN, 2, 128)                             # [dt, c, uc, m]
    w1t = np.zeros((128, 3, 2, 128), np.float16)
    for kt in range(3):
        w1t[0:64, kt] = w1r[2 * kt]
        if 2 * kt + 1 < FS:
            w1t[64:128, kt] = w1r[2 * kt + 1]

    # local weights: [L2, 1280, 256] * a2 -> fp16 -> [core, l, p, kt*2+oc, m]
    lwf = (local_w * a2[None, None, :]).astype(np.float16)
    lwp = lwf.reshape(NCORES, LC, KT2, 128, 2, 128).transpose(0, 1, 3, 2, 4, 5)
    lwp = np.ascontiguousarray(lwp)            # [core, l, p, kt, oc, m]

    # x2: [128, 513, 256] fp16; top=x[c,t], bottom=x[c,t+1]
    xt = np.ascontiguousarray(x.transpose(2, 1, 0)).astype(np.float16)  # [c, t, b]
    x2g = np.zeros((128, L + 1, B), np.float16)
    x2g[0:64, 0:L] = xt
    x2g[64:128, 0:L - 1] = xt[:, 1:L]

    b1_sb = np.ascontiguousarray(bias1.reshape(2, 128).T)          # [p, uc]
    b2_all = bias2.reshape(NCORES, LC, 2, 128).transpose(0, 3, 1, 2)  # [core, p, l, oc]

    in_maps = []
    for c in range(NCORES):
        t0 = LC * c
        x2_c = np.ascontiguousarray(x2g[:, t0: t0 + XCOLS]).reshape(128, XCOLS * B)
        in_maps.append({
            "x2": x2_c,
            "w1t": np.ascontiguousarray(w1t).reshape(128, 3 * 2 * 128),
            "lw": np.ascontiguousarray(lwp[c]).reshape(LC, 128, KT2 * 2 * 128),
            "b1": b1_sb,
            "b2": np.ascontiguousarray(b2_all[c]).reshape(128, LC * 2),
        })
    return in_maps


def kernel(**inputs):
    nc = _build_nc()
    in_maps = _preprocess(**inputs)
    trace = bool(int(os.environ.get("BASS_KERNEL_TRACE", "0")))
    res = bass_utils.run_bass_kernel_spmd(
        nc, in_maps, core_ids=list(range(NCORES)), trace=trace)
    if trace:
        kernel.last_exec_time_ns = res.exec_time_ns
        kernel.last_results = res
    out = np.empty((B, L2, U), np.float32)
    for c in range(NCORES):
        z = res.results[c]["z"].reshape(LC, U, B).astype(np.float32)
        out[:, LC * c: LC * (c + 1), :] = z.transpose(2, 0, 1)
    return out


# revision 9
# speedup vs baseline: 1.6613x; 1.0625x over previous
"""Trainium2 Bass kernel for nn_ConvLocalBlock (Conv1D+BN+ReLU -> LocallyConnected1D+BN+ReLU).

Sharding: sequence-parallel over the L2=504 output positions across 8 cores
(63 positions each), full batch B=256 per core.  Conv weights replicated;
each core computes the y positions (l..l+4 window) it needs locally.

Layouts (host-prepared, fp16 matmul operands, fp32 accumulation):
  x2  [128, 72, 256]  partitions 0:64 = x[c, t], 64:128 = x[c, t+1]  (c-major, b fastest)
  w1t [128, 3, 2, 128] conv weight k-tiles (dt-pairs stacked on partitions), BN1-folded
  lw  [63, 128, 20, 128] per-position local weights as [l, k-part, (kt,oc), m], BN2-folded
  b1  [128, 2]   folded conv bias per (u-part, uc)
  b2  [128, 63, 2] folded local bias per (o-part, l, oc)
Output per core: z [63, 256, 256] fp32 in [l, o, b] layout; host reassembles to [B, L2, U].
"""
import sys
import os

for _p in ('/opt/trn_rl_repo',):
    if _p not in sys.path:
        sys.path.insert(0, _p)

import numpy as np

import concourse.bass as bass
import concourse.tile as tile
import concourse.mybir as mybir
from concourse import bacc, bass_utils

dt = mybir.dt

EPS = 1e-3
FS = 5
B, L, CIN, U = 256, 512, 64, 256
L1 = L - FS + 1            # 508
L2 = L1 - FS + 1           # 504
NCORES = 8
LC = L2 // NCORES          # 63 positions per core
NPOS = LC + FS - 1         # 67 y positions needed per core
XCOLS = NPOS + FS          # 72 x2 columns per core (incl. shifted/zero pad)
KT2 = 10                   # local-stage k tiles (j=0..4  x  uc=0..1)

_NC_CACHE = {}


def _build_nc():
    """Build the single-core Tile program (SPMD across 8 cores)."""
    if 'nc' in _NC_CACHE:
        return _NC_CACHE['nc']
    nc = bacc.Bacc("TRN2", target_bir_lowering=False, debug=False)

    x2_d = nc.dram_tensor("x2", [128, XCOLS * B], dt.float16, kind="ExternalInput")
    w1_d = nc.dram_tensor("w1t", [128, 3 * 2 * 128], dt.float16, kind="ExternalInput")
    lw_d = nc.dram_tensor("lw", [LC, 128, KT2 * 2 * 128], dt.float16, kind="ExternalInput")
    b1_d = nc.dram_tensor("b1", [128, 2], dt.float32, kind="ExternalInput")
    b2_d = nc.dram_tensor("b2", [128, LC * 2], dt.float32, kind="ExternalInput")
    z_d = nc.dram_tensor("z", [LC, U, B], dt.float16, kind="ExternalOutput")

    with tile.TileContext(nc) as tc:
        with tc.tile_pool(name="const", bufs=1) as cpool, \
             tc.tile_pool(name="ybuf", bufs=1) as ypool, \
             tc.tile_pool(name="lwp", bufs=12) as lwpool, \
             tc.tile_pool(name="zp", bufs=12) as zpool, \
             tc.tile_pool(name="ps1", bufs=4, space="PSUM") as ps1, \
             tc.tile_pool(name="ps2", bufs=4, space="PSUM") as ps2:

            x2_t = cpool.tile([128, XCOLS, B], dt.float16)
            w1_t = cpool.tile([128, 3, 2, 128], dt.float16)
            b1_t = cpool.tile([128, 2], dt.float32)
            b2_t = cpool.tile([128, LC, 2], dt.float32)
            nc.sync.dma_start(w1_t[:], w1_d.ap().rearrange("p (k u m) -> p k u m", k=3, u=2))
            x2_src = x2_d.ap().rearrange("p (t b) -> p t b", b=B)
            x2_bounds = [0, 6] + list(range(18, XCOLS, 12)) + [XCOLS]
            nc.sync.dma_start(x2_t[:, 0:6, :], x2_src[:, 0:6, :])
            nc.sync.dma_start(b1_t[:], b1_d.ap()[:])
            for cb, ce in zip(x2_bounds[1:-1], x2_bounds[2:]):
                nc.sync.dma_start(x2_t[:, cb:ce, :], x2_src[:, cb:ce, :])
            nc.sync.dma_start(b2_t[:], b2_d.ap().rearrange("p (l u) -> p l u", u=2))

            y_t = [ypool.tile([128, NPOS * B], dt.float16, tag=f"y{uc}",
                              name=f"y{uc}") for uc in range(2)]

            # ---- stage 1: conv (+BN1+ReLU) into Y[uc][:, t*B : (t+2)*B] ----
            ngroups = (NPOS + 1) // 2          # 34 groups (last single-position)
            for g in range(ngroups):
                npos_g = 2 if 2 * g + 1 < NPOS else 1
                n = npos_g * B
                t0 = 2 * g
                for uc in range(2):
                    ps = ps1.tile([128, 2 * B], dt.float32, tag="convps")
                    for kt in range(3):
                        # k-tile kt reads x2 columns shifted by 2*kt; kt2's
                        # lower 64 weight rows are zero (host-padded), keeping
                        # every matmul a uniform full-array K=128 op.
                        rhs = x2_t[:, t0 + 2 * kt: t0 + 2 * kt + npos_g, :]
                        lhsT = w1_t[:, kt, uc, :]
                        nc.tensor.matmul(ps[:, :n], lhsT, rhs,
                                         start=(kt == 0), stop=(kt == 2))
                    # BN1+ReLU epilogue: relu(x + b1), fp16 out; DVE for uc0, ACT for uc1
                    if uc == 0:
                        nc.vector.tensor_scalar(
                            out=y_t[uc][:, t0 * B: t0 * B + n],
                            in0=ps[:, :n],
                            scalar1=b1_t[:, uc:uc + 1],
                            scalar2=0.0,
                            op0=mybir.AluOpType.add,
                            op1=mybir.AluOpType.max)
                    else:
                        nc.scalar.activation(
                            y_t[uc][:, t0 * B: t0 * B + n], ps[:, :n],
                            mybir.ActivationFunctionType.Relu,
                            bias=b1_t[:, uc:uc + 1], scale=1.0)

            # ---- stage 2: locally-connected (+BN2+ReLU) ----
            for l in range(LC):
                lw_t = lwpool.tile([128, KT2 * 2, 128], dt.float16, tag="lw")
                nc.sync.dma_start(
                    lw_t[:], lw_d.ap()[l].rearrange("p (k m) -> p k m", m=128))
                for oc in range(2):
                    ps = ps2.tile([128, B], dt.float32, tag="locps")
                    for kt in range(KT2):
                        j, uc = kt // 2, kt % 2
                        nc.tensor.matmul(
                            ps[:],
                            lw_t[:, kt * 2 + oc, :],
                            y_t[uc][:, (l + j) * B: (l + j + 1) * B],
                            start=(kt == 0), stop=(kt == KT2 - 1))
                    z_sb = zpool.tile([128, B], dt.float16, tag="z")
                    nc.scalar.activation(
                        z_sb[:], ps[:], mybir.ActivationFunctionType.Relu,
                        bias=b2_t[:, l, oc:oc + 1], scale=1.0)
                    nc.scalar.dma_start(z_d.ap()[l, oc * 128:(oc + 1) * 128, :], z_sb[:])

    nc.compile()
    _NC_CACHE['nc'] = nc
    return nc


def _preprocess(x, conv_w, conv_b, g1, b1, m1, v1, local_w, local_b, g2, b2, m2, v2):
    """Fold BN into weights/biases, build per-core shards in device layouts."""
    f32 = np.float32
    a1 = (g1 / np.sqrt(v1 + EPS)).astype(f32)                      # [U]
    bias1 = ((conv_b - m1) * a1 + b1).astype(f32)                  # [U]
    a2 = (g2 / np.sqrt(v2 + EPS)).astype(f32)                      # [U]
    bias2 = ((local_b - m2[None, :]) * a2[None, :] + b2[None, :]).astype(f32)  # [L2, U]

    w1f = (conv_w * a1[None, None, :]).astype(np.float16)          # [5, 64, 256]
    w1r = w1f.reshape(FS, CIN, 2, 128)                             # [dt, c, uc, m]
    w1t = np.zeros((128, 3, 2, 128), np.float16)
    for kt in range(3):
        w1t[0:64, kt] = w1r[2 * kt]
        if 2 * kt + 1 < FS:
            w1t[64:128, kt] = w1r[2 * kt + 1]

    # local weights: [L2, 1280, 256] * a2 -> fp16 -> [core, l, p, kt*2+oc, m]
    lwf = (local_w * a2[None, None, :]).astype(np.float16)
    lwp = lwf.reshape(NCORES, LC, KT2, 128, 2, 128).transpose(0, 1, 3, 2, 4, 5)
    lwp = np.ascontiguousarray(lwp)            # [core, l, p, kt, oc, m]

    # x2: [128, 513, 256] fp16; top=x[c,t], bottom=x[c,t+1]
    xt = np.ascontiguousarray(x.transpose(2, 1, 0)).astype(np.float16)  # [c, t, b]
    x2g = np.zeros((128, L + 1, B), np.float16)
    x2g[0:64, 0:L] = xt
    x2g[64:128, 0:L - 1] = xt[:, 1:L]

    b1_sb = np.ascontiguousarray(bias1.reshape(2, 128).T)          # [p, uc]
    b2_all = bias2.reshape(NCORES, LC, 2, 128).transpose(0, 3, 1, 2)  # [core, p, l, oc]

    in_maps = []
    for c in range(NCORES):
        t0 = LC * c
        x2_c = np.ascontiguousarray(x2g[:, t0: t0 + XCOLS]).reshape(128, XCOLS * B)
        in_maps.append({
            "x2": x2_c,
            "w1t": np.ascontiguousarray(w1t).reshape(128, 3 * 2 * 128),
            "lw": np.ascontiguousarray(lwp[c]).reshape(LC, 128, KT2 * 2 * 128),
            "b1": b1_sb,
            "b2": np.ascontiguousarray(b2_all[c]).reshape(128, LC * 2),
        })
    return in_maps


def kernel(**inputs):
    nc = _build_nc()
    in_maps = _preprocess(**inputs)
    trace = bool(int(os.environ.get("BASS_KERNEL_TRACE", "0")))
    res = bass_utils.run_bass_kernel_spmd(
        nc, in_maps, core_ids=list(range(NCORES)), trace=trace)
    if trace:
        kernel.last_exec_time_ns = res.exec_time_ns
        kernel.last_results = res
    out = np.empty((B, L2, U), np.float32)
    for c in range(NCORES):
        z = res.results[c]["z"].reshape(LC, U, B).astype(np.float32)
        out[:, LC * c: LC * (c + 1), :] = z.transpose(2, 0, 1)
    return out


# revision 10
# speedup vs baseline: 1.8269x; 1.0997x over previous
"""Trainium2 Bass kernel for nn_ConvLocalBlock (Conv1D+BN+ReLU -> LocallyConnected1D+BN+ReLU).

Sharding: sequence-parallel over the L2=504 output positions across 8 cores
(63 positions each), full batch B=256 per core.  Conv weights replicated;
each core computes the y positions (l..l+4 window) it needs locally.

Layouts (host-prepared, fp16 matmul operands, fp32 accumulation):
  x2  [128, 72, 256]  partitions 0:64 = x[c, t], 64:128 = x[c, t+1]  (c-major, b fastest)
  w1t [128, 3, 2, 128] conv weight k-tiles (dt-pairs stacked on partitions), BN1-folded
  lw  [63, 128, 20, 128] per-position local weights as [l, k-part, (kt,oc), m], BN2-folded
  b1  [128, 2]   folded conv bias per (u-part, uc)
  b2  [128, 63, 2] folded local bias per (o-part, l, oc)
Output per core: z [63, 256, 256] fp32 in [l, o, b] layout; host reassembles to [B, L2, U].
"""
import sys
import os

for _p in ('/opt/trn_rl_repo',):
    if _p not in sys.path:
        sys.path.insert(0, _p)

import numpy as np

import concourse.bass as bass
import concourse.tile as tile
import concourse.mybir as mybir
from concourse import bacc, bass_utils

dt = mybir.dt

EPS = 1e-3
FS = 5
B, L, CIN, U = 256, 512, 64, 256
L1 = L - FS + 1            # 508
L2 = L1 - FS + 1           # 504
NCORES = 8
LC = L2 // NCORES          # 63 positions per core
NPOS = LC + FS - 1         # 67 y positions needed per core
XCOLS = NPOS + FS          # 72 x2 columns per core (incl. shifted/zero pad)
KT2 = 10                   # local-stage k tiles (j=0..4  x  uc=0..1)

_NC_CACHE = {}


def _build_nc():
    """Build the single-core Tile program (SPMD across 8 cores)."""
    if 'nc' in _NC_CACHE:
        return _NC_CACHE['nc']
    nc = bacc.Bacc("TRN2", target_bir_lowering=False, debug=False)

    x2_d = nc.dram_tensor("x2", [128, XCOLS * B], dt.float16, kind="ExternalInput")
    w1_d = nc.dram_tensor("w1t", [128, 3 * 2 * 128], dt.float16, kind="ExternalInput")
    lw_d = nc.dram_tensor("lw", [LC, 128, KT2 * 2 * 128], dt.float16, kind="ExternalInput")
    b1_d = nc.dram_tensor("b1", [128, 2], dt.float32, kind="ExternalInput")
    b2_d = nc.dram_tensor("b2", [128, LC * 2], dt.float32, kind="ExternalInput")
    z_d = nc.dram_tensor("z", [LC, U, B], dt.float16, kind="ExternalOutput")

    with tile.TileContext(nc) as tc:
        with tc.tile_pool(name="const", bufs=1) as cpool, \
             tc.tile_pool(name="ybuf", bufs=1) as ypool, \
             tc.tile_pool(name="lwp", bufs=12) as lwpool, \
             tc.tile_pool(name="zp", bufs=12) as zpool, \
             tc.tile_pool(name="ps1", bufs=4, space="PSUM") as ps1, \
             tc.tile_pool(name="ps2", bufs=4, space="PSUM") as ps2:

            x2_t = cpool.tile([128, XCOLS, B], dt.float16)
            w1_t = cpool.tile([128, 3, 2, 128], dt.float16)
            b1_t = cpool.tile([128, 2], dt.float32)
            b2_t = cpool.tile([128, LC, 2], dt.float32)
            nc.sync.dma_start(w1_t[:], w1_d.ap().rearrange("p (k u m) -> p k u m", k=3, u=2))
            x2_src = x2_d.ap().rearrange("p (t b) -> p t b", b=B)
            x2_bounds = [0, 6] + list(range(18, XCOLS, 12)) + [XCOLS]
            nc.sync.dma_start(x2_t[:, 0:6, :], x2_src[:, 0:6, :])
            nc.sync.dma_start(b1_t[:], b1_d.ap()[:])
            for cb, ce in zip(x2_bounds[1:-1], x2_bounds[2:]):
                nc.sync.dma_start(x2_t[:, cb:ce, :], x2_src[:, cb:ce, :])
            nc.sync.dma_start(b2_t[:], b2_d.ap().rearrange("p (l u) -> p l u", u=2))

            y_t = [ypool.tile([128, NPOS * B], dt.float16, tag=f"y{uc}",
                              name=f"y{uc}") for uc in range(2)]

            # ---- stage 1: conv (+BN1+ReLU) into Y[uc][:, t*B : (t+2)*B] ----
            ngroups = (NPOS + 1) // 2          # 34 groups (last single-position)
            for g in range(ngroups):
                npos_g = 2 if 2 * g + 1 < NPOS else 1
                n = npos_g * B
                t0 = 2 * g
                for uc in range(2):
                    ps = ps1.tile([128, 2 * B], dt.float32, tag="convps")
                    for kt in range(3):
                        # k-tile kt reads x2 columns shifted by 2*kt; kt2's
                        # lower 64 weight rows are zero (host-padded), keeping
                        # every matmul a uniform full-array K=128 op.
                        rhs = x2_t[:, t0 + 2 * kt: t0 + 2 * kt + npos_g, :]
                        lhsT = w1_t[:, kt, uc, :]
                        nc.tensor.matmul(ps[:, :n], lhsT, rhs,
                                         start=(kt == 0), stop=(kt == 2))
                    # BN1+ReLU epilogue: relu(x + b1), fp16 out; DVE for uc0, ACT for uc1
                    if uc == 0:
                        nc.vector.tensor_scalar(
                            out=y_t[uc][:, t0 * B: t0 * B + n],
                            in0=ps[:, :n],
                            scalar1=b1_t[:, uc:uc + 1],
                            scalar2=0.0,
                            op0=mybir.AluOpType.add,
                            op1=mybir.AluOpType.max)
                    else:
                        nc.scalar.activation(
                            y_t[uc][:, t0 * B: t0 * B + n], ps[:, :n],
                            mybir.ActivationFunctionType.Relu,
                            bias=b1_t[:, uc:uc + 1], scale=1.0)

            # ---- stage 2: locally-connected (+BN2+ReLU) ----
            for l in range(LC):
                lw_t = lwpool.tile([128, KT2 * 2, 128], dt.float16, tag="lw")
                nc.sync.dma_start(
                    lw_t[:], lw_d.ap()[l].rearrange("p (k m) -> p k m", m=128))
                for oc in range(2):
                    ps = ps2.tile([128, B], dt.float32, tag="locps")
                    for kt in range(KT2):
                        j, uc = kt // 2, kt % 2
                        nc.tensor.matmul(
                            ps[:],
                            lw_t[:, kt * 2 + oc, :],
                            y_t[uc][:, (l + j) * B: (l + j + 1) * B],
                            start=(kt == 0), stop=(kt == KT2 - 1))
                    z_sb = zpool.tile([128, B], dt.float16, tag="z")
                    nc.vector.tensor_scalar(
                        out=z_sb[:], in0=ps[:],
                        scalar1=b2_t[:, l, oc:oc + 1], scalar2=0.0,
                        op0=mybir.AluOpType.add, op1=mybir.AluOpType.max)
                    nc.scalar.dma_start(z_d.ap()[l, oc * 128:(oc + 1) * 128, :], z_sb[:])

    nc.compile()
    _NC_CACHE['nc'] = nc
    return nc


def _preprocess(x, conv_w, conv_b, g1, b1, m1, v1, local_w, local_b, g2, b2, m2, v2):
    """Fold BN into weights/biases, build per-core shards in device layouts."""
    f32 = np.float32
    a1 = (g1 / np.sqrt(v1 + EPS)).astype(f32)                      # [U]
    bias1 = ((conv_b - m1) * a1 + b1).astype(f32)                  # [U]
    a2 = (g2 / np.sqrt(v2 + EPS)).astype(f32)                      # [U]
    bias2 = ((local_b - m2[None, :]) * a2[None, :] + b2[None, :]).astype(f32)  # [L2, U]

    w1f = (conv_w * a1[None, None, :]).astype(np.float16)          # [5, 64, 256]
    w1r = w1f.reshape(FS, CIN, 2, 128)                             # [dt, c, uc, m]
    w1t = np.zeros((128, 3, 2, 128), np.float16)
    for kt in range(3):
        w1t[0:64, kt] = w1r[2 * kt]
        if 2 * kt + 1 < FS:
            w1t[64:128, kt] = w1r[2 * kt + 1]

    # local weights: [L2, 1280, 256] * a2 -> fp16 -> [core, l, p, kt*2+oc, m]
    lwf = (local_w * a2[None, None, :]).astype(np.float16)
    lwp = lwf.reshape(NCORES, LC, KT2, 128, 2, 128).transpose(0, 1, 3, 2, 4, 5)
    lwp = np.ascontiguousarray(lwp)            # [core, l, p, kt, oc, m]

    # x2: [128, 513, 256] fp16; top=x[c,t], bottom=x[c,t+1]
    xt = np.ascontiguousarray(x.transpose(2, 1, 0)).astype(np.float16)  # [c, t, b]
    x2g = np.zeros((128, L + 1, B), np.float16)
    x2g[0:64, 0:L] = xt
    x2g[64:128, 0:L - 1] = xt[:, 1:L]

    b1_sb = np.ascontiguousarray(bias1.reshape(2, 128).T)          # [p, uc]
    b2_all = bias2.reshape(NCORES, LC, 2, 128).transpose(0, 3, 1, 2)  # [core, p, l, oc]

    in_maps = []
    for c in range(NCORES):
        t0 = LC * c
        x2_c = np.ascontiguousarray(x2g[:, t0: t0 + XCOLS]).reshape(128, XCOLS * B)
        in_maps.append({
            "x2": x2_c,
            "w1t": np.ascontiguousarray(w1t).reshape(128, 3 * 2 * 128),
            "lw": np.ascontiguousarray(lwp[c]).reshape(LC, 128, KT2 * 2 * 128),
            "b1": b1_sb,
            "b2": np.ascontiguousarray(b2_all[c]).reshape(128, LC * 2),
        })
    return in_maps


def kernel(**inputs):
    nc = _build_nc()
    in_maps = _preprocess(**inputs)
    trace = bool(int(os.environ.get("BASS_KERNEL_TRACE", "0")))
    res = bass_utils.run_bass_kernel_spmd(
        nc, in_maps, core_ids=list(range(NCORES)), trace=trace)
    if trace:
        kernel.last_exec_time_ns = res.exec_time_ns
        kernel.last_results = res
    out = np.empty((B, L2, U), np.float32)
    for c in range(NCORES):
        z = res.results[c]["z"].reshape(LC, U, B).astype(np.float32)
        out[:, LC * c: LC * (c + 1), :] = z.transpose(2, 0, 1)
    return out
